# revision 23
# baseline (speedup 1.0000x reference)
"""Bahdanau additive attention kernel for Trainium2 (8 NeuronCores, SPMD).

Problem: hidden [32,1024], encoder_outputs [32,2048,1024], W_w [1024,2048],
W_b [1024], v_w [1,1024], v_b [1] ->
  context [32,1024], weights [32,2048]

  energy  = tanh(hidden @ Wh^T + enc @ We^T + W_b)     (Wh = W_w[:, :H], We = W_w[:, H:])
  scores  = energy @ v_w[0]   (+ v_b, irrelevant to softmax)
  weights = softmax(scores, axis=seq)
  context = weights @ enc

Sharding: data-parallel over batch B across the 8 cores (4 batches/core),
full W/v replicated per core. No cross-core communication.

Per-core dataflow (all matmuls bf16 operands, fp32 PSUM accumulation):
  - W_w / hidden / v cast fp32->bf16 inline during SWDGE DMA (gpsimd).
  - W^T / hidden^T / enc^T produced by xbar DMA-transpose (2-byte dtype);
    a [128, n*128] -> [128, n, 128] transpose yields chunk layout
    out[p, c, j] = in[j, c*128 + p], i.e. natural 128-chunking of the
    contracted dimension onto partitions.
  - enc transposes issue on SyncE (HWDGE), weight transposes on ScalarE
    (HWDGE) so the two streams don't serialize behind each other.
  - Emission order front-loads the We path + the first half-batch of enc
    so the e_proj matmul pipeline starts as early as possible.
  - bias(b, o) = h_proj(b, o) + W_b(o) is fused into the tanh as the
    ScalarE activation per-partition bias (energy laid out [o, s]).
  - scores = sum_o v[o] * energy[o, s] via PSUM-accumulated matmuls.
  - softmax on a single partition row [1, 2048].
  - probs transposed (xbar) to [128, 16] so context = probs^T-weighted
    sum over s runs as PSUM-accumulated matmuls against the resident
    natural-layout bf16 enc tiles.
"""

import numpy as np
from contextlib import ExitStack

import concourse.bass as bass
import concourse.mybir as mybir
import concourse.tile as tile
from concourse import bacc
from concourse.bass_utils import run_bass_kernel_spmd
from concourse.masks import make_identity
from concourse.tile_rust import add_dep_helper

B, S, H = 32, 2048, 1024
NCORES = 8
BL = B // NCORES          # batches per core
HC = H // 128             # h-chunks (contraction) = 8
OC = H // 128             # o-chunks (output feature) = 8
SC = S // 128             # s-chunks per batch = 16
ST = 512                  # matmul moving free-dim tile over s
NST = S // ST             # s-tiles per batch = 4

F32 = mybir.dt.float32
BF16 = mybir.dt.bfloat16
AF = mybir.ActivationFunctionType


def _body(ctx: ExitStack, tc: tile.TileContext, hidden_d, enc_d, ww_d, wb_d,
          vw_d, ctx_d, wts_d):
    nc = tc.nc

    singles = ctx.enter_context(tc.tile_pool(name="singles", bufs=1))
    enc_pool = ctx.enter_context(tc.tile_pool(name="enc_nat", bufs=3))
    encT_pool = ctx.enter_context(tc.tile_pool(name="encT", bufs=2))

    weT = singles.tile([128, HC, H], BF16)   # weT[p,c,o] = We[o, c*128+p]
    # wvhT[:, c, 0] = W_b chunk c; [:, c, 1] = v chunk c; [:, c, 2+b] = hidden^T
    wvhT = singles.tile([128, HC, 16], BF16)
    bias_sb = singles.tile([128, OC, BL], F32)  # bias_sb[p,oc,b] = hproj+W_b

    enc_r = enc_d.rearrange("b (sc p) h -> b p sc h", p=128)
    ww_r = ww_d.rearrange("(oc p) c -> p oc c", p=128)

    def load_enc_quarter(enc_nat, b, q):
        return nc.gpsimd.dma_start(out=enc_nat[:, q * 4:(q + 1) * 4, :],
                                   in_=enc_r[b, :, q * 4:(q + 1) * 4, :])

    half_load = {}

    def transpose_half(enc_nat, encT, half):
        # encT[p, c, s] = enc[b, half*1024 + s, c*128+p]
        last = None
        for scl in range(SC // 2):
            sc = half * (SC // 2) + scl
            last = nc.sync.dma_start(out=encT[:, :, scl * 128:(scl + 1) * 128],
                                     in_=enc_nat[:, sc, :], transpose=True)
        return last

    # ---------------- front-loaded prep ----------------
    # Bias path (Wh -> WhT -> h_proj) first: it feeds the first tanh, and
    # its serial chain (SWDGE cast -> xbar transposes -> PE) is the longest.
    # enc batch-0 quarters + the We path follow immediately so the e_proj
    # matmul stream starts right behind h_proj.
    wprep_cm = tc.tile_pool(name="wprep", bufs=1)
    wprep = wprep_cm.__enter__()
    hps_cm = tc.tile_pool(name="hprep_ps", bufs=4, space="PSUM")
    hps = hps_cm.__enter__()
    wtr_cm = tc.tile_pool(name="wtr_ps", bufs=2, space="PSUM")
    wtr = wtr_cm.__enter__()
    whT = wprep.tile([128, HC, H], BF16)     # whT[p,c,o] = Wh[o, c*128+p]

    ident = wprep.tile([128, 128], BF16)
    make_identity(nc, ident)

    # SWDGE order: the tiny W_b/v/hidden rows, then We, Wh, then enc b0.
    pad16 = wprep.tile([16, H], BF16)
    nc.vector.memset(pad16, 0.0)
    nc.gpsimd.dma_start(out=pad16[0:1, :], in_=wb_d)
    nc.gpsimd.dma_start(out=pad16[1:2, :], in_=vw_d)
    nc.gpsimd.dma_start(out=pad16[2:2 + BL, :], in_=hidden_d)
    w16e = wprep.tile([128, OC, H], BF16)    # w16e[p,oc,h] = We[oc*128+p, h]
    nc.gpsimd.dma_start(out=w16e, in_=ww_r[:, :, H:2 * H])
    w16h = wprep.tile([128, OC, H], BF16)    # w16h[p,oc,h] = Wh[oc*128+p, h]
    nc.gpsimd.dma_start(out=w16h, in_=ww_r[:, :, 0:H])
    enc_nat0 = enc_pool.tile([128, SC, H], BF16, tag="enc_nat")
    load_enc_quarter(enc_nat0, 0, 0)
    load_enc_quarter(enc_nat0, 0, 1)

    # one xbar transpose covers W_b, v and hidden^T at once
    nc.sync.dma_start(out=wvhT, in_=pad16, transpose=True)
    wbf = singles.tile([128, HC], F32)       # W_b chunks as fp32 scalars
    nc.vector.tensor_copy(out=wbf, in_=wvhT[:, :, 0])

    # HAM warmup: dense junk matmuls so the PE clock is at 8/8 when real
    # work lands (PE-transpose mode does not count as HAM activity).
    warm = wprep.tile([128, 512], BF16)
    nc.vector.memset(warm, 0.0)
    warm_ps = hps.tile([1, 512], F32, tag="warm", bufs=1)
    for _ in range(14):
        nc.tensor.matmul(warm_ps, lhsT=warm[:, 0:1], rhs=warm,
                         start=True, stop=True, skip_group_check=True)

    def pe_transpose_w(w16, wT):
        # wT[:, c, o] = w16[o-part, c-free] blocks, transposed on the PE
        for c in range(HC):
            for g in range(2):
                ps = wtr.tile([128, 512], BF16, tag="wtr")
                for i in range(4):
                    oc = g * 4 + i
                    nc.tensor.transpose(out=ps[:, i * 128:(i + 1) * 128],
                                        in_=w16[:, oc, c * 128:(c + 1) * 128],
                                        identity=ident)
                nc.vector.tensor_copy(
                    out=wT[:, c, g * 512:(g + 1) * 512], in_=ps)

    pe_transpose_w(w16e, weT)
    pe_transpose_w(w16h, whT)

    for oc in range(OC):
        hp = hps.tile([128, BL], F32)
        for c in range(HC):
            nc.tensor.matmul(hp, lhsT=whT[:, c, oc * 128:(oc + 1) * 128],
                             rhs=wvhT[:, c, 2:2 + BL],
                             start=(c == 0), stop=(c == HC - 1))
        nc.vector.tensor_scalar_add(out=bias_sb[:, oc, :], in0=hp,
                                    scalar1=wbf[:, oc:oc + 1])
    encT00 = encT_pool.tile([128, HC, S // 2], BF16, tag="encT")
    tr_last = transpose_half(enc_nat0, encT00, 0)

    # rest of enc batch 0, ordered after the first transposes so SWDGE
    # copies and xbar transposes alternate instead of interleaving (each
    # xbar-mode flip drains the in-flight DMA burst).
    i2 = load_enc_quarter(enc_nat0, 0, 2)
    add_dep_helper(i2.ins, tr_last.ins, reason="phase: b0 q2 after encT00")
    b0q3 = load_enc_quarter(enc_nat0, 0, 3)

    # second warmup burst right before the e_proj stream (the xbar/PE
    # transposes above don't count as HAM activity, so the clock may have
    # dropped back to 4/8 by now)
    for _ in range(8):
        nc.tensor.matmul(warm_ps, lhsT=warm[:, 0:1], rhs=warm,
                         start=True, stop=True, skip_group_check=True)
    wtr_cm.__exit__(None, None, None)
    hps_cm.__exit__(None, None, None)
    wprep_cm.__exit__(None, None, None)

    warm16 = singles.tile([128, 512], BF16)
    nc.vector.memset(warm16, 0.0)

    # zeroed staging rows for the probs transpose (junk rows stay 0 forever)
    p16a = singles.tile([16, S], BF16)
    p16b = singles.tile([16, S], BF16)
    nc.vector.memset(p16a, 0.0)
    nc.vector.memset(p16b, 0.0)

    # ---------------- main-loop pools (reuse prep address space) ----------
    energy_pool = ctx.enter_context(tc.tile_pool(name="energy", bufs=10))
    sm_pool = ctx.enter_context(tc.tile_pool(name="sm", bufs=2))
    p16_pool = ctx.enter_context(tc.tile_pool(name="p16", bufs=2))
    out_pool = ctx.enter_context(tc.tile_pool(name="outs", bufs=2))
    eproj_ps = ctx.enter_context(tc.tile_pool(name="eproj_ps", bufs=2, space="PSUM"))
    scores_ps = ctx.enter_context(tc.tile_pool(name="scores_ps", bufs=2, space="PSUM"))
    ctx_ps = ctx.enter_context(tc.tile_pool(name="ctx_ps", bufs=1, space="PSUM"))

    # ---------------- main loop over local batches ----------------
    for b in range(BL):
        if b == 0:
            enc_nat = enc_nat0
        else:
            enc_nat = enc_pool.tile([128, SC, H], BF16, tag="enc_nat")
            for hq in range(2):
                ld = nc.gpsimd.dma_start(out=enc_nat[:, hq * 8:(hq + 1) * 8, :],
                                         in_=enc_r[b, :, hq * 8:(hq + 1) * 8, :])
                # alternate SWDGE-load / xbar-transpose phases (b1 runs free:
                # batch 0's pipeline is still ramping and the chain would
                # starve b1's transposes)
                add_dep_helper(ld.ins, tr_last.ins,
                               reason="phase: loads after transposes")
                half_load[(b, hq)] = ld

        scores_sb = sm_pool.tile([1, S], F32)
        pm4 = sm_pool.tile([1, NST], F32)

        for half in range(2):
            if b == 0 and half == 0:
                encT = encT00
            elif b == 0:
                # batch-0 ramp: quarter-granularity load/transpose phasing
                encT = encT_pool.tile([128, HC, S // 2], BF16, tag="encT")
                for scl in range(SC // 2):
                    sc = SC // 2 + scl
                    tr = nc.sync.dma_start(
                        out=encT[:, :, scl * 128:(scl + 1) * 128],
                        in_=enc_nat[:, sc, :], transpose=True)
                    if scl == 4:
                        add_dep_helper(b0q3.ins, tr_last.ins,
                                       reason="phase: b0 q3 after sc8-11")
                    tr_last = tr
            else:
                encT = encT_pool.tile([128, HC, S // 2], BF16, tag="encT")
                tr0 = transpose_half(enc_nat, encT, half)
                tr_last = tr0
                if (b, 1) in half_load:
                    # don't start this half's transposes until both of this
                    # batch's load phases are off the SDMA engines
                    add_dep_helper(tr0.ins, half_load[(b, 1)].ins,
                                   reason="phase: transposes after loads")

            for stl in range(NST // 2):
                st = half * (NST // 2) + stl
                s0 = stl * ST
                sc_tile = scores_ps.tile([1, ST], F32, tag="sc_tile")
                en_tiles = []
                for ocp in range(OC // 2):
                    ep = eproj_ps.tile([128, 2, ST], F32)
                    for half_oc in range(2):
                        oc = ocp * 2 + half_oc
                        for c in range(HC):
                            nc.tensor.matmul(ep[:, half_oc, :],
                                             lhsT=weT[:, c, oc * 128:(oc + 1) * 128],
                                             rhs=encT[:, c, s0:s0 + ST],
                                             start=(c == 0), stop=(c == HC - 1))
                    for half_oc in range(2):
                        oc = ocp * 2 + half_oc
                        en = energy_pool.tile([128, ST], BF16, tag="en")
                        nc.scalar.activation(out=en, in_=ep[:, half_oc, :],
                                             func=AF.Tanh,
                                             bias=bias_sb[:, oc, b:b + 1])
                        en_tiles.append(en)
                # scores in a second phase: by the time the PE reaches these,
                # every tanh has long finished -> no per-group sem stall
                for oc in range(OC):
                    nc.tensor.matmul(sc_tile, lhsT=wvhT[:, oc, 1:2],
                                     rhs=en_tiles[oc],
                                     start=(oc == 0), stop=(oc == OC - 1))
                nc.scalar.copy(out=scores_sb[0:1, st * ST:(st + 1) * ST],
                               in_=sc_tile)
                nc.vector.reduce_max(out=pm4[0:1, st:st + 1],
                                     in_=scores_sb[0:1, st * ST:(st + 1) * ST],
                                     axis=mybir.AxisListType.X)

        # softmax tail: exp straight to bf16 with fused row-sum, context
        # matmuls on unnormalized probs, 1/sum applied on the PSUM drain.
        neg_m = sm_pool.tile([1, 1], F32)
        nc.vector.reduce_max(out=neg_m, in_=pm4, axis=mybir.AxisListType.X,
                             negate=True)
        probs16 = p16a if b % 2 == 0 else p16b
        ssum = sm_pool.tile([1, 1], F32)
        nc.scalar.activation(out=probs16[0:1, :], in_=scores_sb, func=AF.Exp,
                             bias=neg_m, accum_out=ssum)
        rinv = sm_pool.tile([1, 1], F32)
        nc.vector.reciprocal(out=rinv, in_=ssum)
        probsT = p16_pool.tile([128, SC, 16], BF16)  # probsT[p,c,0] = p[c*128+p]
        nc.sync.dma_start(out=probsT, in_=probs16, transpose=True)

        if b == BL - 1:
            wsc = scores_ps.tile([1, ST], F32, tag="sc_tile")
            for _ in range(20):
                nc.tensor.matmul(wsc, lhsT=warm16[:, 0:1], rhs=warm16,
                                 start=True, stop=True, skip_group_check=True)

        ctxp = ctx_ps.tile([1, H], F32)
        for c in range(SC):
            for h2 in range(2):
                nc.tensor.matmul(ctxp[0:1, h2 * 512:(h2 + 1) * 512],
                                 lhsT=probsT[:, c, 0:1],
                                 rhs=enc_nat[:, c, h2 * 512:(h2 + 1) * 512],
                                 start=(c == 0), stop=(c == SC - 1))
        ctx_sb = out_pool.tile([1, H], F32)
        nc.scalar.activation(out=ctx_sb, in_=ctxp, func=AF.Identity,
                             scale=rinv, bias=0.0)
        nc.sync.dma_start(out=ctx_d[b:b + 1, :], in_=ctx_sb)

        # fp32 weights output, off the critical path
        wts_f = out_pool.tile([1, S], F32)
        nc.scalar.activation(out=wts_f, in_=scores_sb, func=AF.Exp, bias=neg_m)
        nc.vector.tensor_scalar_mul(out=wts_f, in0=wts_f, scalar1=rinv)
        nc.sync.dma_start(out=wts_d[b:b + 1, :], in_=wts_f)


def build():
    nc = bacc.Bacc("TRN2", target_bir_lowering=False, debug=False,
                   enable_asserts=False, num_devices=NCORES)
    hidden_d = nc.dram_tensor("hidden", [BL, H], F32, kind="ExternalInput").ap()
    enc_d = nc.dram_tensor("enc", [BL, S, H], F32, kind="ExternalInput").ap()
    ww_d = nc.dram_tensor("w_w", [H, 2 * H], F32, kind="ExternalInput").ap()
    wb_d = nc.dram_tensor("w_b", [H], F32, kind="ExternalInput").ap()
    vw_d = nc.dram_tensor("v_w", [1, H], F32, kind="ExternalInput").ap()
    ctx_d = nc.dram_tensor("ctx", [BL, H], F32, kind="ExternalOutput").ap()
    wts_d = nc.dram_tensor("wts", [BL, S], F32, kind="ExternalOutput").ap()

    with tile.TileContext(nc) as tc:
        with ExitStack() as stack:
            _body(stack, tc, hidden_d, enc_d, ww_d, wb_d, vw_d, ctx_d, wts_d)
    nc.compile()
    return nc


_CACHE: dict = {}


def get_nc():
    if "nc" not in _CACHE:
        _CACHE["nc"] = build()
    return _CACHE["nc"]


def make_in_maps(hidden, encoder_outputs, W_w, W_b, v_w):
    hidden = np.asarray(hidden, dtype=np.float32)
    enc = np.asarray(encoder_outputs, dtype=np.float32)
    ww = np.ascontiguousarray(np.asarray(W_w, dtype=np.float32))
    wb = np.ascontiguousarray(np.asarray(W_b, dtype=np.float32))
    vw = np.ascontiguousarray(np.asarray(v_w, dtype=np.float32))
    in_maps = []
    for core in range(NCORES):
        sl = slice(core * BL, (core + 1) * BL)
        in_maps.append({
            "hidden": np.ascontiguousarray(hidden[sl]),
            "enc": np.ascontiguousarray(enc[sl]),
            "w_w": ww,
            "w_b": wb,
            "v_w": vw,
        })
    return in_maps


def kernel(hidden, encoder_outputs, W_w, W_b, v_w, v_b):
    nc = get_nc()
    in_maps = make_in_maps(hidden, encoder_outputs, W_w, W_b, v_w)
    res = run_bass_kernel_spmd(nc, in_maps, core_ids=list(range(NCORES)))
    ctx = np.concatenate([res.results[c]["ctx"] for c in range(NCORES)], axis=0)
    wts = np.concatenate([res.results[c]["wts"] for c in range(NCORES)], axis=0)
    return ctx.astype(np.float32), wts.astype(np.float32)


# revision 24
# speedup vs baseline: 1.1550x; 1.1550x over previous
"""Bahdanau additive attention kernel for Trainium2 (8 NeuronCores, SPMD).

Problem: hidden [32,1024], encoder_outputs [32,2048,1024], W_w [1024,2048],
W_b [1024], v_w [1,1024], v_b [1] ->
  context [32,1024], weights [32,2048]

  energy  = tanh(hidden @ Wh^T + enc @ We^T + W_b)     (Wh = W_w[:, :H], We = W_w[:, H:])
  scores  = energy @ v_w[0]   (+ v_b, irrelevant to softmax)
  weights = softmax(scores, axis=seq)
  context = weights @ enc

Sharding: data-parallel over batch B across the 8 cores (4 batches/core),
full W/v replicated per core. No cross-core communication.

Per-core dataflow (all matmuls bf16 operands, fp32 PSUM accumulation):
  - W_w / hidden / v cast fp32->bf16 inline during SWDGE DMA (gpsimd).
  - W^T / hidden^T / enc^T produced by xbar DMA-transpose (2-byte dtype);
    a [128, n*128] -> [128, n, 128] transpose yields chunk layout
    out[p, c, j] = in[j, c*128 + p], i.e. natural 128-chunking of the
    contracted dimension onto partitions.
  - enc transposes issue on SyncE (HWDGE), weight transposes on ScalarE
    (HWDGE) so the two streams don't serialize behind each other.
  - Emission order front-loads the We path + the first half-batch of enc
    so the e_proj matmul pipeline starts as early as possible.
  - bias(b, o) = h_proj(b, o) + W_b(o) is fused into the tanh as the
    ScalarE activation per-partition bias (energy laid out [o, s]).
  - scores = sum_o v[o] * energy[o, s] via PSUM-accumulated matmuls.
  - softmax on a single partition row [1, 2048].
  - probs transposed (xbar) to [128, 16] so context = probs^T-weighted
    sum over s runs as PSUM-accumulated matmuls against the resident
    natural-layout bf16 enc tiles.
"""

import numpy as np
from contextlib import ExitStack

import concourse.bass as bass
import concourse.mybir as mybir
import concourse.tile as tile
from concourse import bacc
from concourse.bass_utils import run_bass_kernel_spmd
from concourse.masks import make_identity
from concourse.tile_rust import add_dep_helper

B, S, H = 32, 2048, 1024
NCORES = 8
BL = B // NCORES          # batches per core
HC = H // 128             # h-chunks (contraction) = 8
OC = H // 128             # o-chunks (output feature) = 8
SC = S // 128             # s-chunks per batch = 16
ST = 512                  # matmul moving free-dim tile over s
NST = S // ST             # s-tiles per batch = 4

F32 = mybir.dt.float32
BF16 = mybir.dt.bfloat16
AF = mybir.ActivationFunctionType


def _body(ctx: ExitStack, tc: tile.TileContext, hidden_d, enc_d, ww_d, wb_d,
          vw_d, ctx_d, wts_d):
    nc = tc.nc

    singles = ctx.enter_context(tc.tile_pool(name="singles", bufs=1))
    enc_pool = ctx.enter_context(tc.tile_pool(name="enc_nat", bufs=3))
    encT_pool = ctx.enter_context(tc.tile_pool(name="encT", bufs=2))

    weT = singles.tile([128, HC, H], BF16)   # weT[p,c,o] = We[o, c*128+p]
    # wvhT[:, c, 0] = W_b chunk c; [:, c, 1] = v chunk c; [:, c, 2+b] = hidden^T
    wvhT = singles.tile([128, HC, 16], BF16)
    bias_sb = singles.tile([128, OC, BL], F32)  # bias_sb[p,oc,b] = hproj+W_b

    enc_r = enc_d.rearrange("b (sc p) h -> b p sc h", p=128)
    ww_r = ww_d.rearrange("(oc p) c -> p oc c", p=128)

    def load_enc_quarter(enc_nat, b, q):
        return nc.gpsimd.dma_start(out=enc_nat[:, q * 4:(q + 1) * 4, :],
                                   in_=enc_r[b, :, q * 4:(q + 1) * 4, :])

    half_load = {}

    def transpose_half(enc_nat, encT, half):
        # encT[p, c, s] = enc[b, half*1024 + s, c*128+p]
        last = None
        for scl in range(SC // 2):
            sc = half * (SC // 2) + scl
            last = nc.sync.dma_start(out=encT[:, :, scl * 128:(scl + 1) * 128],
                                     in_=enc_nat[:, sc, :], transpose=True)
        return last

    # ---------------- front-loaded prep ----------------
    # Bias path (Wh -> WhT -> h_proj) first: it feeds the first tanh, and
    # its serial chain (SWDGE cast -> xbar transposes -> PE) is the longest.
    # enc batch-0 quarters + the We path follow immediately so the e_proj
    # matmul stream starts right behind h_proj.
    wprep_cm = tc.tile_pool(name="wprep", bufs=1)
    wprep = wprep_cm.__enter__()
    hps_cm = tc.tile_pool(name="hprep_ps", bufs=4, space="PSUM")
    hps = hps_cm.__enter__()
    wtr_cm = tc.tile_pool(name="wtr_ps", bufs=2, space="PSUM")
    wtr = wtr_cm.__enter__()
    whT = wprep.tile([128, HC, H], BF16)     # whT[p,c,o] = Wh[o, c*128+p]

    ident = wprep.tile([128, 128], BF16)
    make_identity(nc, ident)

    # SWDGE order: the tiny W_b/v/hidden rows, then We, Wh, then enc b0.
    pad16 = wprep.tile([16, H], BF16)
    nc.vector.memset(pad16, 0.0)
    nc.gpsimd.dma_start(out=pad16[0:1, :], in_=wb_d)
    nc.gpsimd.dma_start(out=pad16[1:2, :], in_=vw_d)
    nc.gpsimd.dma_start(out=pad16[2:2 + BL, :], in_=hidden_d)
    w16e = wprep.tile([128, OC, H], BF16)    # w16e[p,oc,h] = We[oc*128+p, h]
    nc.gpsimd.dma_start(out=w16e, in_=ww_r[:, :, H:2 * H])
    w16h = wprep.tile([128, OC, H], BF16)    # w16h[p,oc,h] = Wh[oc*128+p, h]
    nc.gpsimd.dma_start(out=w16h, in_=ww_r[:, :, 0:H])
    enc_nat0 = enc_pool.tile([128, SC, H], BF16, tag="enc_nat")
    load_enc_quarter(enc_nat0, 0, 0)
    load_enc_quarter(enc_nat0, 0, 1)

    # one xbar transpose covers W_b, v and hidden^T at once
    nc.sync.dma_start(out=wvhT, in_=pad16, transpose=True)
    wbf = singles.tile([128, HC], F32)       # W_b chunks as fp32 scalars
    nc.vector.tensor_copy(out=wbf, in_=wvhT[:, :, 0])

    # HAM warmup: dense junk matmuls so the PE clock is at 8/8 when real
    # work lands (PE-transpose mode does not count as HAM activity).
    warm = wprep.tile([128, 512], BF16)
    nc.vector.memset(warm, 0.0)
    warm_ps = hps.tile([1, 512], F32, tag="warm", bufs=1)
    for _ in range(14):
        nc.tensor.matmul(warm_ps, lhsT=warm[:, 0:1], rhs=warm,
                         start=True, stop=True, skip_group_check=True)

    def pe_transpose_w(w16, wT):
        # wT[:, c, o] = w16[o-part, c-free] blocks, transposed on the PE
        for c in range(HC):
            for g in range(2):
                ps = wtr.tile([128, 512], BF16, tag="wtr")
                for i in range(4):
                    oc = g * 4 + i
                    nc.tensor.transpose(out=ps[:, i * 128:(i + 1) * 128],
                                        in_=w16[:, oc, c * 128:(c + 1) * 128],
                                        identity=ident)
                nc.vector.tensor_copy(
                    out=wT[:, c, g * 512:(g + 1) * 512], in_=ps)

    pe_transpose_w(w16e, weT)
    pe_transpose_w(w16h, whT)

    for oc in range(OC):
        hp = hps.tile([128, BL], F32)
        for c in range(HC):
            nc.tensor.matmul(hp, lhsT=whT[:, c, oc * 128:(oc + 1) * 128],
                             rhs=wvhT[:, c, 2:2 + BL],
                             start=(c == 0), stop=(c == HC - 1))
        nc.vector.tensor_scalar_add(out=bias_sb[:, oc, :], in0=hp,
                                    scalar1=wbf[:, oc:oc + 1])
    encT00 = encT_pool.tile([128, HC, S // 2], BF16, tag="encT")
    tr_last = transpose_half(enc_nat0, encT00, 0)

    # rest of enc batch 0, ordered after the first transposes so SWDGE
    # copies and xbar transposes alternate instead of interleaving (each
    # xbar-mode flip drains the in-flight DMA burst).
    i2 = load_enc_quarter(enc_nat0, 0, 2)
    add_dep_helper(i2.ins, tr_last.ins, reason="phase: b0 q2 after encT00")
    b0q3 = load_enc_quarter(enc_nat0, 0, 3)

    # second warmup burst right before the e_proj stream (the xbar/PE
    # transposes above don't count as HAM activity, so the clock may have
    # dropped back to 4/8 by now)
    for _ in range(8):
        nc.tensor.matmul(warm_ps, lhsT=warm[:, 0:1], rhs=warm,
                         start=True, stop=True, skip_group_check=True)
    wtr_cm.__exit__(None, None, None)
    hps_cm.__exit__(None, None, None)
    wprep_cm.__exit__(None, None, None)

    warm16 = singles.tile([128, 512], BF16)
    nc.vector.memset(warm16, 0.0)

    # zeroed staging rows for the probs transpose (junk rows stay 0 forever)
    p16a = singles.tile([16, S], BF16)
    p16b = singles.tile([16, S], BF16)
    nc.vector.memset(p16a, 0.0)
    nc.vector.memset(p16b, 0.0)

    # ---------------- main-loop pools (reuse prep address space) ----------
    energy_pool = ctx.enter_context(tc.tile_pool(name="energy", bufs=10))
    sm_pool = ctx.enter_context(tc.tile_pool(name="sm", bufs=2))
    p16_pool = ctx.enter_context(tc.tile_pool(name="p16", bufs=2))
    out_pool = ctx.enter_context(tc.tile_pool(name="outs", bufs=2))
    eproj_ps = ctx.enter_context(tc.tile_pool(name="eproj_ps", bufs=4, space="PSUM"))
    scores_ps = ctx.enter_context(tc.tile_pool(name="scores_ps", bufs=2, space="PSUM"))
    ctx_ps = ctx.enter_context(tc.tile_pool(name="ctx_ps", bufs=1, space="PSUM"))

    # ---------------- main loop over local batches ----------------
    for b in range(BL):
        if b == 0:
            enc_nat = enc_nat0
        else:
            enc_nat = enc_pool.tile([128, SC, H], BF16, tag="enc_nat")
            for hq in range(2):
                ld = nc.gpsimd.dma_start(out=enc_nat[:, hq * 8:(hq + 1) * 8, :],
                                         in_=enc_r[b, :, hq * 8:(hq + 1) * 8, :])
                # alternate SWDGE-load / xbar-transpose phases (b1 runs free:
                # batch 0's pipeline is still ramping and the chain would
                # starve b1's transposes)
                add_dep_helper(ld.ins, tr_last.ins,
                               reason="phase: loads after transposes")
                half_load[(b, hq)] = ld

        scores_sb = sm_pool.tile([1, S], F32)
        pm4 = sm_pool.tile([1, NST], F32)

        for half in range(2):
            if b == 0 and half == 0:
                encT = encT00
            elif b == 0:
                # batch-0 ramp: quarter-granularity load/transpose phasing
                encT = encT_pool.tile([128, HC, S // 2], BF16, tag="encT")
                for scl in range(SC // 2):
                    sc = SC // 2 + scl
                    tr = nc.sync.dma_start(
                        out=encT[:, :, scl * 128:(scl + 1) * 128],
                        in_=enc_nat[:, sc, :], transpose=True)
                    if scl == 4:
                        add_dep_helper(b0q3.ins, tr_last.ins,
                                       reason="phase: b0 q3 after sc8-11")
                    tr_last = tr
            else:
                encT = encT_pool.tile([128, HC, S // 2], BF16, tag="encT")
                tr0 = transpose_half(enc_nat, encT, half)
                tr_last = tr0
                if (b, 1) in half_load:
                    # don't start this half's transposes until both of this
                    # batch's load phases are off the SDMA engines
                    add_dep_helper(tr0.ins, half_load[(b, 1)].ins,
                                   reason="phase: transposes after loads")

            for stl in range(NST // 2):
                st = half * (NST // 2) + stl
                s0 = stl * ST
                sc_tile = scores_ps.tile([1, ST], F32, tag="sc_tile")
                en_tiles = []
                for oc in range(OC):
                    ep = eproj_ps.tile([128, ST], F32)
                    for c in range(HC):
                        nc.tensor.matmul(ep,
                                         lhsT=weT[:, c, oc * 128:(oc + 1) * 128],
                                         rhs=encT[:, c, s0:s0 + ST],
                                         start=(c == 0), stop=(c == HC - 1))
                    en = energy_pool.tile([128, ST], BF16, tag="en")
                    nc.scalar.activation(out=en, in_=ep, func=AF.Tanh,
                                         bias=bias_sb[:, oc, b:b + 1])
                    en_tiles.append(en)
                # scores in a second phase: by the time the PE reaches these,
                # every tanh has long finished -> no per-group sem stall
                for oc in range(OC):
                    nc.tensor.matmul(sc_tile, lhsT=wvhT[:, oc, 1:2],
                                     rhs=en_tiles[oc],
                                     start=(oc == 0), stop=(oc == OC - 1))
                nc.scalar.copy(out=scores_sb[0:1, st * ST:(st + 1) * ST],
                               in_=sc_tile)
                nc.vector.reduce_max(out=pm4[0:1, st:st + 1],
                                     in_=scores_sb[0:1, st * ST:(st + 1) * ST],
                                     axis=mybir.AxisListType.X)

        # softmax tail: exp straight to bf16 with fused row-sum, context
        # matmuls on unnormalized probs, 1/sum applied on the PSUM drain.
        neg_m = sm_pool.tile([1, 1], F32)
        nc.vector.reduce_max(out=neg_m, in_=pm4, axis=mybir.AxisListType.X,
                             negate=True)
        probs16 = p16a if b % 2 == 0 else p16b
        ssum = sm_pool.tile([1, 1], F32)
        nc.scalar.activation(out=probs16[0:1, :], in_=scores_sb, func=AF.Exp,
                             bias=neg_m, accum_out=ssum)
        rinv = sm_pool.tile([1, 1], F32)
        nc.vector.reciprocal(out=rinv, in_=ssum)
        probsT = p16_pool.tile([128, SC, 16], BF16)  # probsT[p,c,0] = p[c*128+p]
        nc.sync.dma_start(out=probsT, in_=probs16, transpose=True)

        if b == BL - 1:
            wsc = scores_ps.tile([1, ST], F32, tag="sc_tile")
            for _ in range(20):
                nc.tensor.matmul(wsc, lhsT=warm16[:, 0:1], rhs=warm16,
                                 start=True, stop=True, skip_group_check=True)

        ctxp = ctx_ps.tile([1, H], F32)
        for c in range(SC):
            for h2 in range(2):
                nc.tensor.matmul(ctxp[0:1, h2 * 512:(h2 + 1) * 512],
                                 lhsT=probsT[:, c, 0:1],
                                 rhs=enc_nat[:, c, h2 * 512:(h2 + 1) * 512],
                                 start=(c == 0), stop=(c == SC - 1))
        ctx_sb = out_pool.tile([1, H], F32)
        nc.scalar.activation(out=ctx_sb, in_=ctxp, func=AF.Identity,
                             scale=rinv, bias=0.0)
        nc.sync.dma_start(out=ctx_d[b:b + 1, :], in_=ctx_sb)

        # fp32 weights output, off the critical path
        wts_f = out_pool.tile([1, S], F32)
        nc.scalar.activation(out=wts_f, in_=scores_sb, func=AF.Exp, bias=neg_m)
        nc.vector.tensor_scalar_mul(out=wts_f, in0=wts_f, scalar1=rinv)
        nc.sync.dma_start(out=wts_d[b:b + 1, :], in_=wts_f)


def build():
    nc = bacc.Bacc("TRN2", target_bir_lowering=False, debug=False,
                   enable_asserts=False, num_devices=NCORES)
    hidden_d = nc.dram_tensor("hidden", [BL, H], F32, kind="ExternalInput").ap()
    enc_d = nc.dram_tensor("enc", [BL, S, H], F32, kind="ExternalInput").ap()
    ww_d = nc.dram_tensor("w_w", [H, 2 * H], F32, kind="ExternalInput").ap()
    wb_d = nc.dram_tensor("w_b", [H], F32, kind="ExternalInput").ap()
    vw_d = nc.dram_tensor("v_w", [1, H], F32, kind="ExternalInput").ap()
    ctx_d = nc.dram_tensor("ctx", [BL, H], F32, kind="ExternalOutput").ap()
    wts_d = nc.dram_tensor("wts", [BL, S], F32, kind="ExternalOutput").ap()

    with tile.TileContext(nc) as tc:
        with ExitStack() as stack:
            _body(stack, tc, hidden_d, enc_d, ww_d, wb_d, vw_d, ctx_d, wts_d)
    nc.compile()
    return nc


_CACHE: dict = {}


def get_nc():
    if "nc" not in _CACHE:
        _CACHE["nc"] = build()
    return _CACHE["nc"]


def make_in_maps(hidden, encoder_outputs, W_w, W_b, v_w):
    hidden = np.asarray(hidden, dtype=np.float32)
    enc = np.asarray(encoder_outputs, dtype=np.float32)
    ww = np.ascontiguousarray(np.asarray(W_w, dtype=np.float32))
    wb = np.ascontiguousarray(np.asarray(W_b, dtype=np.float32))
    vw = np.ascontiguousarray(np.asarray(v_w, dtype=np.float32))
    in_maps = []
    for core in range(NCORES):
        sl = slice(core * BL, (core + 1) * BL)
        in_maps.append({
            "hidden": np.ascontiguousarray(hidden[sl]),
            "enc": np.ascontiguousarray(enc[sl]),
            "w_w": ww,
            "w_b": wb,
            "v_w": vw,
        })
    return in_maps


def kernel(hidden, encoder_outputs, W_w, W_b, v_w, v_b):
    nc = get_nc()
    in_maps = make_in_maps(hidden, encoder_outputs, W_w, W_b, v_w)
    res = run_bass_kernel_spmd(nc, in_maps, core_ids=list(range(NCORES)))
    ctx = np.concatenate([res.results[c]["ctx"] for c in range(NCORES)], axis=0)
    wts = np.concatenate([res.results[c]["wts"] for c in range(NCORES)], axis=0)
    return ctx.astype(np.float32), wts.astype(np.float32)
